# revision 3
# baseline (speedup 1.0000x reference)
"""HSTUBlockPreprocessor Trainium2 kernel.

Reference semantics (per sample b of B=128):
    out[b] = [ctx0[b]; ctx1[b]; interleave(item[b], action[b])]
where interleave alternates item/action rows token by token, and
out_len[b] = 2 + 2*item_lengths[b].

This is pure data movement, so the kernel is a handful of large strided
DRAM->DRAM DMAs per core. Sharding: data-parallel over the batch dim,
16 samples per core (sample rows are sample-contiguous, so each core's
inputs/outputs are contiguous row-slices of the full tensors).
"""

import numpy as np

import concourse.bass as bass
import concourse.mybir as mybir
from concourse.ap import AP
from concourse import bass_utils

N_CORES = 8
B = 128
D = 256

_compiled_cache = {}


def _arith_stride(xs):
    """If xs is an arithmetic progression return its stride, else None.
    A single element counts (stride 0)."""
    if len(xs) <= 1:
        return 0
    d = xs[1] - xs[0]
    for i in range(2, len(xs)):
        if xs[i] - xs[i - 1] != d:
            return None
    return d


def _emit_group_dmas(sync, sem, src_tensor, src_offs, dst_tensor, dst_offs,
                     length, dst_row_step):
    """DMA `len(src_offs)` blocks of [length, D] rows from src_tensor (row
    offsets src_offs, contiguous rows) into dst_tensor at row offsets
    dst_offs with dst rows strided by dst_row_step rows. Merges into a
    single 3-dim-AP DMA when the offsets are arithmetic progressions.
    Returns number of dma_start calls issued (each .then_inc(sem, 16))."""
    n = len(src_offs)
    ss = _arith_stride(src_offs)
    ds = _arith_stride(dst_offs)
    if n > 1 and ss is not None and ds is not None:
        src_ap = AP(src_tensor, src_offs[0] * D,
                    [[ss * D, n], [D, length], [1, D]])
        dst_ap = AP(dst_tensor, dst_offs[0] * D,
                    [[ds * D, n], [dst_row_step * D, length], [1, D]])
        sync.dma_start(out=dst_ap, in_=src_ap).then_inc(sem, 16)
        return 1
    cnt = 0
    for so, do in zip(src_offs, dst_offs):
        src_ap = AP(src_tensor, so * D, [[D, length], [1, D]])
        dst_ap = AP(dst_tensor, do * D,
                    [[dst_row_step * D, length], [1, D]])
        sync.dma_start(out=dst_ap, in_=src_ap).then_inc(sem, 16)
        cnt += 1
    return cnt


def _build_kernel(local_lengths, reps=1):
    """Build the per-core Bass program. local_lengths: tuple of 16 ints,
    identical on every core (asserted by caller). reps>1 repeats the whole
    DMA program back-to-back (serialized) for slope-based benchmarking."""
    spc = len(local_lengths)          # samples per core
    n_rows = int(sum(local_lengths))  # item rows per core
    out_rows = 2 * spc + 2 * n_rows   # output rows per core

    ioff = np.concatenate([[0], np.cumsum(local_lengths)]).astype(np.int64)
    ooff = (2 * np.arange(spc + 1) + 2 * ioff).astype(np.int64)

    nc = bass.Bass(trn_type="TRN2")
    item = nc.dram_tensor("item", [n_rows, D], mybir.dt.float32,
                          kind="ExternalInput")
    action = nc.dram_tensor("action", [n_rows, D], mybir.dt.float32,
                            kind="ExternalInput")
    ctx0 = nc.dram_tensor("ctx0", [spc, D], mybir.dt.float32,
                          kind="ExternalInput")
    ctx1 = nc.dram_tensor("ctx1", [spc, D], mybir.dt.float32,
                          kind="ExternalInput")
    lengths = nc.dram_tensor("lengths", [spc], mybir.dt.int32,
                             kind="ExternalInput")
    out = nc.dram_tensor("out", [out_rows, D], mybir.dt.float32,
                         kind="ExternalOutput")
    out_len = nc.dram_tensor("out_len", [spc], mybir.dt.int32,
                             kind="ExternalOutput")

    # Group samples by length so each group's row blocks can merge into one
    # strided-AP DMA (for the alternating 512/1536 pattern: 2 groups).
    groups = {}
    for s, length in enumerate(local_lengths):
        groups.setdefault(int(length), []).append(s)

    with (
        nc.sbuf_tensor([1, spc], mybir.dt.int32) as len_in,
        nc.sbuf_tensor([1, spc], mybir.dt.int32) as len_out,
        nc.semaphore("dma_sem") as dma_sem,
        nc.semaphore("len_in_sem") as len_in_sem,
        nc.semaphore("len_done_sem") as len_done_sem,
        nc.Block() as block,
    ):
        n_dma = [0]

        @block.sync
        def _(sync):
            # lengths -> SBUF for the out_len computation
            sync.dma_start(out=len_in[:1, :], in_=lengths[None, :]) \
                .then_inc(len_in_sem, 16)

            for _rep in range(reps):
                # ctx0 / ctx1 rows: sample s -> out row ooff[s] (+1 ctx1)
                all_samples = list(range(spc))
                ctx_dst = [int(ooff[s]) for s in all_samples]
                n_dma[0] += _emit_group_dmas(
                    sync, dma_sem, ctx0, all_samples, out, ctx_dst, 1, 1)
                n_dma[0] += _emit_group_dmas(
                    sync, dma_sem, ctx1, all_samples, out,
                    [o + 1 for o in ctx_dst], 1, 1)

                # item/action blocks, grouped by sample length
                for length, samples in groups.items():
                    src_offs = [int(ioff[s]) for s in samples]
                    item_dst = [int(ooff[s]) + 2 for s in samples]
                    n_dma[0] += _emit_group_dmas(
                        sync, dma_sem, item, src_offs, out, item_dst,
                        length, 2)
                    n_dma[0] += _emit_group_dmas(
                        sync, dma_sem, action, src_offs, out,
                        [o + 1 for o in item_dst], length, 2)
                # serialize iterations so reps scale linearly
                sync.wait_ge(dma_sem, 16 * n_dma[0])

            # out_len = 2*lengths + 2 (computed on DVE) -> DRAM
            sync.wait_ge(len_done_sem, 1)
            sync.dma_start(out=out_len[None, :], in_=len_out[:1, :]) \
                .then_inc(dma_sem, 16)
            sync.wait_ge(dma_sem, 16 * (n_dma[0] + 1))

        @block.vector
        def _(vector):
            vector.wait_ge(len_in_sem, 16)
            vector.tensor_scalar(
                len_out[:1, :], len_in[:1, :], 2, 2,
                mybir.AluOpType.mult, mybir.AluOpType.add,
            ).then_inc(len_done_sem, 1)

    return nc


def kernel(item_values, action_values, ctx0_values, ctx1_values,
           item_lengths, ctx0_lengths, ctx1_lengths):
    item_values = np.asarray(item_values, dtype=np.float32)
    action_values = np.asarray(action_values, dtype=np.float32)
    ctx0_values = np.asarray(ctx0_values, dtype=np.float32)
    ctx1_values = np.asarray(ctx1_values, dtype=np.float32)
    item_lengths = np.asarray(item_lengths, dtype=np.int32)
    assert np.all(np.asarray(ctx0_lengths) == 1)
    assert np.all(np.asarray(ctx1_lengths) == 1)

    b = item_lengths.shape[0]
    assert b % N_CORES == 0
    spc = b // N_CORES

    # Per-core sharding: core c gets samples [spc*c, spc*(c+1)). SPMD needs
    # every core to see the same local length pattern (APs are compiled in).
    per_core = item_lengths.reshape(N_CORES, spc)
    assert (per_core == per_core[0]).all(), (
        "SPMD kernel requires identical per-core length patterns")
    local_lengths = tuple(int(x) for x in per_core[0])
    n_rows = int(sum(local_lengths))
    out_rows = 2 * spc + 2 * n_rows

    if local_lengths not in _compiled_cache:
        _compiled_cache[local_lengths] = _build_kernel(local_lengths)
    nc = _compiled_cache[local_lengths]

    in_maps = []
    for c in range(N_CORES):
        in_maps.append({
            "item": item_values[c * n_rows:(c + 1) * n_rows],
            "action": action_values[c * n_rows:(c + 1) * n_rows],
            "ctx0": ctx0_values[c * spc:(c + 1) * spc],
            "ctx1": ctx1_values[c * spc:(c + 1) * spc],
            "lengths": item_lengths[c * spc:(c + 1) * spc],
        })

    res = bass_utils.run_bass_kernel_spmd(nc, in_maps, list(range(N_CORES)))
    results = res.results

    out_vals = np.concatenate([r["out"] for r in results], axis=0)
    out_len = np.concatenate([r["out_len"] for r in results], axis=0)
    assert out_vals.shape == (2 * b + 2 * item_lengths.sum(), D)
    return out_vals, out_len.astype(np.int32)


# revision 5
# speedup vs baseline: 3.7969x; 3.7969x over previous
"""HSTUBlockPreprocessor Trainium2 kernel.

Reference semantics (per sample b of B=128):
    out[b] = [ctx0[b]; ctx1[b]; interleave(item[b], action[b])]
where interleave alternates item/action rows token by token, and
out_len[b] = 2 + 2*item_lengths[b].

This is pure data movement, so the kernel is a handful of large strided
DRAM->DRAM DMAs per core. Sharding: data-parallel over the batch dim,
16 samples per core (sample rows are sample-contiguous, so each core's
inputs/outputs are contiguous row-slices of the full tensors).
"""

import numpy as np

import concourse.bass as bass
import concourse.mybir as mybir
from concourse.ap import AP
from concourse import bass_utils

N_CORES = 8
B = 128
D = 256

_compiled_cache = {}


def _arith_stride(xs):
    """If xs is an arithmetic progression return its stride, else None.
    A single element counts (stride 0)."""
    if len(xs) <= 1:
        return 0
    d = xs[1] - xs[0]
    for i in range(2, len(xs)):
        if xs[i] - xs[i - 1] != d:
            return None
    return d


def _emit_group_dmas(sync, sem, src_tensor, src_offs, dst_tensor, dst_offs,
                     length, dst_row_step):
    """DMA `len(src_offs)` blocks of [length, D] rows from src_tensor (row
    offsets src_offs, contiguous rows) into dst_tensor at row offsets
    dst_offs with dst rows strided by dst_row_step rows. Merges into a
    single 3-dim-AP DMA when the offsets are arithmetic progressions.
    Returns number of dma_start calls issued (each .then_inc(sem, 16))."""
    n = len(src_offs)
    ss = _arith_stride(src_offs)
    ds = _arith_stride(dst_offs)
    if n > 1 and ss is not None and ds is not None:
        src_ap = AP(src_tensor, src_offs[0] * D,
                    [[ss * D, n], [D, length], [1, D]])
        dst_ap = AP(dst_tensor, dst_offs[0] * D,
                    [[ds * D, n], [dst_row_step * D, length], [1, D]])
        sync.dma_start(out=dst_ap, in_=src_ap).then_inc(sem, 16)
        return 1
    cnt = 0
    for so, do in zip(src_offs, dst_offs):
        src_ap = AP(src_tensor, so * D, [[D, length], [1, D]])
        dst_ap = AP(dst_tensor, do * D,
                    [[dst_row_step * D, length], [1, D]])
        sync.dma_start(out=dst_ap, in_=src_ap).then_inc(sem, 16)
        cnt += 1
    return cnt


def _build_kernel(local_lengths, reps=1):
    """Build the per-core Bass program. local_lengths: tuple of 16 ints,
    identical on every core (asserted by caller). reps>1 repeats the whole
    DMA program back-to-back (serialized) for slope-based benchmarking."""
    spc = len(local_lengths)          # samples per core
    n_rows = int(sum(local_lengths))  # item rows per core
    out_rows = 2 * spc + 2 * n_rows   # output rows per core

    ioff = np.concatenate([[0], np.cumsum(local_lengths)]).astype(np.int64)
    ooff = (2 * np.arange(spc + 1) + 2 * ioff).astype(np.int64)

    nc = bass.Bass(trn_type="TRN2")
    item = nc.dram_tensor("item", [n_rows, D], mybir.dt.float32,
                          kind="ExternalInput")
    action = nc.dram_tensor("action", [n_rows, D], mybir.dt.float32,
                            kind="ExternalInput")
    ctx0 = nc.dram_tensor("ctx0", [spc, D], mybir.dt.float32,
                          kind="ExternalInput")
    ctx1 = nc.dram_tensor("ctx1", [spc, D], mybir.dt.float32,
                          kind="ExternalInput")
    lengths = nc.dram_tensor("lengths", [spc], mybir.dt.int32,
                             kind="ExternalInput")
    out = nc.dram_tensor("out", [out_rows, D], mybir.dt.float32,
                         kind="ExternalOutput")
    out_len = nc.dram_tensor("out_len", [spc], mybir.dt.int32,
                             kind="ExternalOutput")

    with (
        nc.sbuf_tensor([1, spc], mybir.dt.int32) as len_in,
        nc.sbuf_tensor([1, spc], mybir.dt.int32) as len_out,
        nc.semaphore("dma_sem") as dma_sem,
        nc.semaphore("len_in_sem") as len_in_sem,
        nc.semaphore("len_done_sem") as len_done_sem,
        nc.Block() as block,
    ):
        n_dma = [0]

        @block.sync
        def _(sync):
            # lengths -> SBUF for the out_len computation
            sync.dma_start(out=len_in[:1, :], in_=lengths[None, :]) \
                .then_inc(len_in_sem, 16)

            # One 2-dim-AP DMA per (tensor, sample). Keeping the outer AP
            # dim = rows lets the DGE round-robin descriptors across all
            # 16 SDMA engines; a merged 3-dim AP (outer dim = samples)
            # measured 4.5x slower (532us vs 119us per exec).
            for _rep in range(reps):
                for s in range(spc):
                    length = int(local_lengths[s])
                    io, oo = int(ioff[s]), int(ooff[s])
                    sync.dma_start(
                        out=AP(out, oo * D, [[D, 1], [1, D]]),
                        in_=AP(ctx0, s * D, [[D, 1], [1, D]]),
                    ).then_inc(dma_sem, 16)
                    sync.dma_start(
                        out=AP(out, (oo + 1) * D, [[D, 1], [1, D]]),
                        in_=AP(ctx1, s * D, [[D, 1], [1, D]]),
                    ).then_inc(dma_sem, 16)
                    sync.dma_start(
                        out=AP(out, (oo + 2) * D, [[2 * D, length], [1, D]]),
                        in_=AP(item, io * D, [[D, length], [1, D]]),
                    ).then_inc(dma_sem, 16)
                    sync.dma_start(
                        out=AP(out, (oo + 3) * D, [[2 * D, length], [1, D]]),
                        in_=AP(action, io * D, [[D, length], [1, D]]),
                    ).then_inc(dma_sem, 16)
                    n_dma[0] += 4
                # serialize iterations so reps scale linearly
                sync.wait_ge(dma_sem, 16 * n_dma[0])

            # out_len = 2*lengths + 2 (computed on DVE) -> DRAM
            sync.wait_ge(len_done_sem, 1)
            sync.dma_start(out=out_len[None, :], in_=len_out[:1, :]) \
                .then_inc(dma_sem, 16)
            sync.wait_ge(dma_sem, 16 * (n_dma[0] + 1))

        @block.vector
        def _(vector):
            vector.wait_ge(len_in_sem, 16)
            vector.tensor_scalar(
                len_out[:1, :], len_in[:1, :], 2, 2,
                mybir.AluOpType.mult, mybir.AluOpType.add,
            ).then_inc(len_done_sem, 1)

    return nc


def kernel(item_values, action_values, ctx0_values, ctx1_values,
           item_lengths, ctx0_lengths, ctx1_lengths):
    item_values = np.asarray(item_values, dtype=np.float32)
    action_values = np.asarray(action_values, dtype=np.float32)
    ctx0_values = np.asarray(ctx0_values, dtype=np.float32)
    ctx1_values = np.asarray(ctx1_values, dtype=np.float32)
    item_lengths = np.asarray(item_lengths, dtype=np.int32)
    assert np.all(np.asarray(ctx0_lengths) == 1)
    assert np.all(np.asarray(ctx1_lengths) == 1)

    b = item_lengths.shape[0]
    assert b % N_CORES == 0
    spc = b // N_CORES

    # Per-core sharding: core c gets samples [spc*c, spc*(c+1)). SPMD needs
    # every core to see the same local length pattern (APs are compiled in).
    per_core = item_lengths.reshape(N_CORES, spc)
    assert (per_core == per_core[0]).all(), (
        "SPMD kernel requires identical per-core length patterns")
    local_lengths = tuple(int(x) for x in per_core[0])
    n_rows = int(sum(local_lengths))
    out_rows = 2 * spc + 2 * n_rows

    if local_lengths not in _compiled_cache:
        _compiled_cache[local_lengths] = _build_kernel(local_lengths)
    nc = _compiled_cache[local_lengths]

    in_maps = []
    for c in range(N_CORES):
        in_maps.append({
            "item": item_values[c * n_rows:(c + 1) * n_rows],
            "action": action_values[c * n_rows:(c + 1) * n_rows],
            "ctx0": ctx0_values[c * spc:(c + 1) * spc],
            "ctx1": ctx1_values[c * spc:(c + 1) * spc],
            "lengths": item_lengths[c * spc:(c + 1) * spc],
        })

    res = bass_utils.run_bass_kernel_spmd(nc, in_maps, list(range(N_CORES)))
    results = res.results

    out_vals = np.concatenate([r["out"] for r in results], axis=0)
    out_len = np.concatenate([r["out_len"] for r in results], axis=0)
    assert out_vals.shape == (2 * b + 2 * item_lengths.sum(), D)
    return out_vals, out_len.astype(np.int32)
